# revision 1
# baseline (speedup 1.0000x reference)
"""Trainium2 Bass kernel for nn_ItemAgg (GNN message passing).

Strategy: shard edges by destination user across 8 cores (users split into 8
contiguous ranges of 12500) -> zero cross-core communication; each core
computes the full output rows for its users.

Per core, edges are sorted by local user id and grouped into 128-user blocks;
each block's edge list is padded to NT subtiles of 128 edges.  Device pipeline
per 128-edge subtile:
  gather item/rating/user rows (indirect DMA, f32) -> PE transpose to
  feature-major -> bf16 MLP chain (gv, attention) with N=512 batched matmuls
  -> exp on ScalarE -> one-hot scatter-matmul accumulating [128 users, 65]
  (h numerator cols 0:64, softmax denominator col 64) in PSUM over the block
  -> normalize, final Linear, DMA out.

Softmax is computed without per-segment max subtraction: softmax is
shift-invariant, logits here are O(0.1), so exp() is numerically safe.
"""

import os
import sys

import numpy as np

sys.path.insert(0, "/opt/trn_rl_repo")

import concourse.bass as bass
import concourse.bacc as bacc
import concourse.mybir as mybir
import concourse.tile as tile
from concourse.bass_utils import run_bass_kernel_spmd
from concourse.masks import make_identity

U, I, E, D, R = 100000, 50000, 2000000, 64, 5
NCORES = 8
UPC = U // NCORES            # users per core
NBLK = (UPC + 127) // 128    # 128-user blocks per core
BF16 = mybir.dt.bfloat16
F32 = mybir.dt.float32
I32 = mybir.dt.int32


def _host_shard(row_idxs, col_idxs, rating):
    """Sort/pad edges per core into [NBLK, 128, NT] index planes."""
    row_idxs = np.asarray(row_idxs, dtype=np.int64)
    col_idxs = np.asarray(col_idxs, dtype=np.int64)
    rating = np.asarray(rating, dtype=np.int64)

    per_core = []
    max_sub = 1
    for c in range(NCORES):
        base = c * UPC
        sel = (col_idxs >= base) & (col_idxs < base + UPC)
        it = row_idxs[sel]
        rt = rating[sel]
        loc = col_idxs[sel] - base
        order = np.argsort(loc, kind="stable")
        it, rt, loc = it[order], rt[order], loc[order]
        blk = loc >> 7
        cnt = np.bincount(blk, minlength=NBLK)
        max_sub = max(max_sub, int(((cnt + 127) // 128).max()))
        per_core.append((it, rt, loc, blk, cnt))

    NT = ((max_sub + 3) // 4) * 4  # multiple of 4 for N=512 matmul groups

    shards = []
    for c in range(NCORES):
        it, rt, loc, blk, cnt = per_core[c]
        base = c * UPC
        cap = NT * 128
        it_p = np.zeros((NBLK, cap), dtype=np.int32)
        rt_p = np.zeros((NBLK, cap), dtype=np.int32)
        us_p = np.full((NBLK, cap), base, dtype=np.int32)
        rl_p = np.full((NBLK, cap), 300.0, dtype=np.float32)
        starts = np.concatenate(([0], np.cumsum(cnt)))
        for b in range(NBLK):
            s, n = starts[b], cnt[b]
            it_p[b, :n] = it[s : s + n]
            rt_p[b, :n] = rt[s : s + n]
            us_p[b, :n] = loc[s : s + n] + base
            rl_p[b, :n] = (loc[s : s + n] & 127).astype(np.float32)
        # [NBLK, cap] -> [NBLK, NT, 128] -> [NBLK, 128, NT] so SBUF tile
        # [128, NT] has subtile t in column t.
        shards.append(
            dict(
                it=np.ascontiguousarray(it_p.reshape(NBLK, NT, 128).transpose(0, 2, 1)),
                rt=np.ascontiguousarray(rt_p.reshape(NBLK, NT, 128).transpose(0, 2, 1)),
                us=np.ascontiguousarray(us_p.reshape(NBLK, NT, 128).transpose(0, 2, 1)),
                rl=np.ascontiguousarray(rl_p.reshape(NBLK, NT, 128).transpose(0, 2, 1)),
            )
        )
    return NT, shards


def _build_program(NT):
    nc = bacc.Bacc(None, target_bir_lowering=False, debug=False)
    bf = mybir.dt.np(BF16)

    # --- I/O declarations ---
    it_idx = nc.declare_dram_parameter("it_idx", [NBLK, 128, NT], I32, isOutput=False)
    rt_idx = nc.declare_dram_parameter("rt_idx", [NBLK, 128, NT], I32, isOutput=False)
    us_idx = nc.declare_dram_parameter("us_idx", [NBLK, 128, NT], I32, isOutput=False)
    rel = nc.declare_dram_parameter("rel", [NBLK, 128, NT], F32, isOutput=False)
    item_t = nc.declare_dram_parameter("item_feat", [I, D], F32, isOutput=False)
    user_t = nc.declare_dram_parameter("user_feat", [U, D], F32, isOutput=False)
    rate_t = nc.declare_dram_parameter("rating_feat", [R, D], F32, isOutput=False)
    w_gv1 = nc.declare_dram_parameter("w_gv1", [2 * D, D], BF16, isOutput=False)
    w_gv2 = nc.declare_dram_parameter("w_gv2", [D, D], BF16, isOutput=False)
    w_at1a = nc.declare_dram_parameter("w_at1a", [D, D], BF16, isOutput=False)
    w_at1b = nc.declare_dram_parameter("w_at1b", [D, D], BF16, isOutput=False)
    w_at2 = nc.declare_dram_parameter("w_at2", [D, D], BF16, isOutput=False)
    w_at3 = nc.declare_dram_parameter("w_at3", [D, 1], BF16, isOutput=False)
    w_out = nc.declare_dram_parameter("w_out", [D, D], BF16, isOutput=False)
    b_gv1 = nc.declare_dram_parameter("b_gv1", [D, 1], F32, isOutput=False)
    b_gv2 = nc.declare_dram_parameter("b_gv2", [D, 1], F32, isOutput=False)
    b_at1 = nc.declare_dram_parameter("b_at1", [D, 1], F32, isOutput=False)
    b_at2 = nc.declare_dram_parameter("b_at2", [D, 1], F32, isOutput=False)
    b3c = nc.declare_dram_parameter("b3c", [128, 1], F32, isOutput=False)
    wb_t = nc.declare_dram_parameter("wb_t", [128, D], F32, isOutput=False)
    iota_r = nc.declare_dram_parameter("iota_r", [128, 128], F32, isOutput=False)
    out = nc.declare_dram_parameter("out", [NBLK * 128, D], F32, isOutput=True)

    with tile.TileContext(nc) as tc:
        with (
            tc.tile_pool(name="const", bufs=1) as cp,
            tc.tile_pool(name="idx", bufs=2) as ip,
            tc.tile_pool(name="gath", bufs=6) as gp,
            tc.tile_pool(name="work", bufs=3) as wp,
            tc.tile_pool(name="mlp", bufs=3, space="PSUM") as pm,
            tc.tile_pool(name="tr", bufs=2, space="PSUM") as pt,
            tc.tile_pool(name="sc", bufs=1, space="PSUM") as ps,
            tc.tile_pool(name="misc", bufs=2, space="PSUM") as px,
        ):
            # constants
            id_f = cp.tile([128, 128], F32, tag="id_f")
            make_identity(nc, id_f[:])
            id_b = cp.tile([128, 128], BF16, tag="id_b")
            nc.vector.tensor_copy(id_b[:], id_f[:])
            c_iota = cp.tile([128, 128], F32, tag="c_iota")
            nc.sync.dma_start(c_iota[:], iota_r[:])
            c_wgv1 = cp.tile([128, D], BF16, tag="c_wgv1")
            nc.sync.dma_start(c_wgv1[:], w_gv1[:])
            c_wgv2 = cp.tile([D, D], BF16, tag="c_wgv2")
            nc.sync.dma_start(c_wgv2[:], w_gv2[:])
            c_wat1a = cp.tile([D, D], BF16, tag="c_wat1a")
            nc.sync.dma_start(c_wat1a[:], w_at1a[:])
            c_wat1b = cp.tile([D, D], BF16, tag="c_wat1b")
            nc.sync.dma_start(c_wat1b[:], w_at1b[:])
            c_wat2 = cp.tile([D, D], BF16, tag="c_wat2")
            nc.sync.dma_start(c_wat2[:], w_at2[:])
            c_wat3 = cp.tile([D, 1], BF16, tag="c_wat3")
            nc.sync.dma_start(c_wat3[:], w_at3[:])
            c_wout = cp.tile([D, D], BF16, tag="c_wout")
            nc.sync.dma_start(c_wout[:], w_out[:])
            c_bgv1 = cp.tile([D, 1], F32, tag="c_bgv1")
            nc.sync.dma_start(c_bgv1[:], b_gv1[:])
            c_bgv2 = cp.tile([D, 1], F32, tag="c_bgv2")
            nc.sync.dma_start(c_bgv2[:], b_gv2[:])
            c_bat1 = cp.tile([D, 1], F32, tag="c_bat1")
            nc.sync.dma_start(c_bat1[:], b_at1[:])
            c_bat2 = cp.tile([D, 1], F32, tag="c_bat2")
            nc.sync.dma_start(c_bat2[:], b_at2[:])
            c_b3 = cp.tile([128, 1], F32, tag="c_b3")
            nc.sync.dma_start(c_b3[:], b3c[:])
            c_wb = cp.tile([128, D], F32, tag="c_wb")
            nc.sync.dma_start(c_wb[:], wb_t[:])

            for b in range(NBLK):
                t_it = ip.tile([128, NT], I32, tag="t_it")
                nc.sync.dma_start(t_it[:], it_idx[b])
                t_rt = ip.tile([128, NT], I32, tag="t_rt")
                nc.sync.dma_start(t_rt[:], rt_idx[b])
                t_us = ip.tile([128, NT], I32, tag="t_us")
                nc.sync.dma_start(t_us[:], us_idx[b])
                t_rl = ip.tile([128, NT], F32, tag="t_rl")
                nc.sync.dma_start(t_rl[:], rel[b])

                acc = ps.tile([128, D + 1], F32, tag="acc")

                for g in range(NT // 4):
                    XR = wp.tile([128, 512], BF16, tag="XR")
                    XU = wp.tile([D, 512], BF16, tag="XU")
                    AUx = wp.tile([D, 512], BF16, tag="AUx")
                    Ss = []
                    for k in range(4):
                        t = g * 4 + k
                        sl = slice(k * 128, (k + 1) * 128)
                        g2 = gp.tile([128, 128], F32, tag="g2")
                        nc.gpsimd.indirect_dma_start(
                            out=g2[:, 0:D],
                            out_offset=None,
                            in_=item_t[:],
                            in_offset=bass.IndirectOffsetOnAxis(
                                ap=t_it[:, t : t + 1], axis=0
                            ),
                        )
                        nc.gpsimd.indirect_dma_start(
                            out=g2[:, D:128],
                            out_offset=None,
                            in_=rate_t[:],
                            in_offset=bass.IndirectOffsetOnAxis(
                                ap=t_rt[:, t : t + 1], axis=0
                            ),
                        )
                        gu = gp.tile([128, D], F32, tag="gu")
                        nc.gpsimd.indirect_dma_start(
                            out=gu[:],
                            out_offset=None,
                            in_=user_t[:],
                            in_offset=bass.IndirectOffsetOnAxis(
                                ap=t_us[:, t : t + 1], axis=0
                            ),
                        )
                        pst = pt.tile([128, 128], F32, tag="trp")
                        nc.tensor.transpose(pst[:], g2[:], id_f[:])
                        nc.scalar.copy(XR[:, sl], pst[:])
                        psu = pt.tile([128, 128], F32, tag="trp")
                        nc.tensor.transpose(psu[0:D, :], gu[:], id_f[:])
                        nc.scalar.copy(XU[:, sl], psu[0:D, :])
                        S = gp.tile([128, 128], BF16, tag="S")
                        nc.vector.tensor_tensor(
                            S[:], c_iota[:],
                            t_rl[:, t : t + 1].to_broadcast([128, 128]),
                            mybir.AluOpType.is_equal,
                        )
                        Ss.append(S)

                    h1p = pm.tile([D, 512], F32, tag="mlpp")
                    nc.tensor.matmul(h1p[:], c_wgv1[:], XR[:], start=True, stop=True)
                    h1s = wp.tile([D, 512], BF16, tag="h1s")
                    nc.scalar.activation(
                        h1s[:], h1p[:], mybir.ActivationFunctionType.Relu,
                        bias=c_bgv1[:],
                    )
                    xp = pm.tile([D, 512], F32, tag="mlpp")
                    nc.tensor.matmul(xp[:], c_wgv2[:], h1s[:], start=True, stop=True)
                    nc.scalar.activation(
                        AUx[:], xp[:], mybir.ActivationFunctionType.Relu,
                        bias=c_bgv2[:],
                    )
                    a1p = pm.tile([D, 512], F32, tag="mlpp")
                    nc.tensor.matmul(a1p[:], c_wat1a[:], AUx[:], start=True, stop=False)
                    nc.tensor.matmul(a1p[:], c_wat1b[:], XU[:], start=False, stop=True)
                    a1s = wp.tile([D, 512], BF16, tag="a1s")
                    nc.scalar.activation(
                        a1s[:], a1p[:], mybir.ActivationFunctionType.Relu,
                        bias=c_bat1[:],
                    )
                    a2p = pm.tile([D, 512], F32, tag="mlpp")
                    nc.tensor.matmul(a2p[:], c_wat2[:], a1s[:], start=True, stop=True)
                    a2s = wp.tile([D, 512], BF16, tag="a2s")
                    nc.scalar.activation(
                        a2s[:], a2p[:], mybir.ActivationFunctionType.Relu,
                        bias=c_bat2[:],
                    )

                    for k in range(4):
                        t = g * 4 + k
                        sl = slice(k * 128, (k + 1) * 128)
                        wlp = px.tile([128, 128], F32, tag="miscp")
                        nc.tensor.matmul(
                            wlp[:, 0:1], a2s[:, sl], c_wat3[:], start=True, stop=True
                        )
                        p = gp.tile([128, 1], F32, tag="p")
                        nc.scalar.activation(
                            p[:], wlp[:, 0:1], mybir.ActivationFunctionType.Exp,
                            bias=c_b3[:],
                        )
                        xtp = px.tile([128, 128], BF16, tag="miscp")
                        nc.tensor.transpose(
                            xtp[:, 0:D], AUx[:, sl], id_b[0:D, 0:D]
                        )
                        rs = gp.tile([128, D + 1], BF16, tag="rs")
                        nc.vector.tensor_tensor(
                            rs[:, 0:D], xtp[:, 0:D], p[:].to_broadcast([128, D]),
                            mybir.AluOpType.mult,
                        )
                        nc.vector.tensor_copy(rs[:, D : D + 1], p[:])
                        nc.tensor.matmul(
                            acc[:], Ss[k][:], rs[:],
                            start=(t == 0), stop=(t == NT - 1),
                        )

                # block finalize
                s_eps = gp.tile([128, 1], F32, tag="s_eps")
                nc.vector.tensor_scalar_add(s_eps[:], acc[:, D : D + 1], 1e-30)
                rcp = gp.tile([128, 1], F32, tag="rcp")
                nc.vector.reciprocal(rcp[:], s_eps[:])
                hn = wp.tile([128, D], BF16, tag="hn")
                nc.vector.tensor_tensor(
                    hn[:], acc[:, 0:D], rcp[:].to_broadcast([128, D]),
                    mybir.AluOpType.mult,
                )
                htp = px.tile([128, 128], BF16, tag="miscp")
                nc.tensor.transpose(htp[0:D, :], hn[:], id_b[:])
                hts = wp.tile([D, 128], BF16, tag="hts")
                nc.scalar.copy(hts[:], htp[0:D, :])
                outp = px.tile([128, 128], F32, tag="miscp")
                nc.tensor.matmul(
                    outp[:, 0:D], hts[:], c_wout[:], start=True, stop=True
                )
                outs = wp.tile([128, D], F32, tag="outs")
                nc.vector.tensor_tensor(
                    outs[:], outp[:, 0:D], c_wb[:], mybir.AluOpType.add
                )
                nc.sync.dma_start(out[b * 128 : (b + 1) * 128, :], outs[:])

    nc.compile()
    return nc


def kernel(**inputs):
    rowi = np.asarray(inputs["row_idxs"])
    coli = np.asarray(inputs["col_idxs"])
    rati = np.asarray(inputs["rating"])
    NT, shards = _host_shard(rowi, coli, rati)

    nc = _build_program(NT)
    bf = mybir.dt.np(BF16)

    def f32(x):
        return np.ascontiguousarray(np.asarray(x, dtype=np.float32))

    common = dict(
        item_feat=f32(inputs["item_feat"]),
        user_feat=f32(inputs["user_feat"]),
        rating_feat=f32(inputs["rating_feat"]),
        w_gv1=f32(inputs["gv_w1"]).astype(bf),
        w_gv2=f32(inputs["gv_w2"]).astype(bf),
        w_at1a=f32(inputs["att_w1"])[:64].astype(bf),
        w_at1b=f32(inputs["att_w1"])[64:].astype(bf),
        w_at2=f32(inputs["att_w2"]).astype(bf),
        w_at3=f32(inputs["att_w3"]).astype(bf),
        w_out=f32(inputs["w_w"]).astype(bf),
        b_gv1=f32(inputs["gv_b1"]).reshape(D, 1),
        b_gv2=f32(inputs["gv_b2"]).reshape(D, 1),
        b_at1=f32(inputs["att_b1"]).reshape(D, 1),
        b_at2=f32(inputs["att_b2"]).reshape(D, 1),
        b3c=np.full((128, 1), np.float32(np.asarray(inputs["att_b3"]).reshape(-1)[0]),
                    dtype=np.float32),
        wb_t=np.tile(f32(inputs["w_b"]).reshape(1, D), (128, 1)),
        iota_r=np.tile(np.arange(128, dtype=np.float32), (128, 1)),
    )
    in_maps = []
    for c in range(NCORES):
        m = dict(common)
        m["it_idx"] = shards[c]["it"]
        m["rt_idx"] = shards[c]["rt"]
        m["us_idx"] = shards[c]["us"]
        m["rel"] = shards[c]["rl"]
        in_maps.append(m)

    trace = os.environ.get("ITEMAGG_TRACE") == "1"
    res = run_bass_kernel_spmd(nc, in_maps, list(range(NCORES)), trace=trace)
    global LAST_RESULT
    LAST_RESULT = res
    outs = [res.results[c]["out"][:UPC] for c in range(NCORES)]
    return np.concatenate(outs, axis=0).astype(np.float32)


LAST_RESULT = None

if __name__ == "__main__":
    pass

